# revision 8
# baseline (speedup 1.0000x reference)
"""Trainium2 Bass kernel: dwconv-QKV attention block, data-parallel over batch on 8 cores."""
import sys

sys.path.insert(0, "/opt/trn_rl_repo")

import numpy as np
import ml_dtypes

from concourse import bass, bacc, mybir, tile
from concourse.bass_utils import run_bass_kernel_spmd
from concourse.masks import make_identity

F32 = mybir.dt.float32
BF16 = mybir.dt.bfloat16
NCORES = 8
B, T, C, HEADS = 16, 1025, 768, 12
BL = B // NCORES  # batches per core
EPS = 1e-5
SCALE = float(C) ** -0.5
LCH = [(0, 512), (512, 1024), (1024, 1025)]  # l-dim chunks (<=512 per PSUM bank)


def _build(nc):
    x = nc.declare_dram_parameter("x", [BL, T, C], F32, isOutput=False)
    diag = nc.declare_dram_parameter("diag", [3, 6, 9, 128, 128], BF16, isOutput=False)
    biasp = nc.declare_dram_parameter("biasp", [3, 6, 128], F32, isOutput=False)
    pwt = nc.declare_dram_parameter("pwt", [12, 64, C], BF16, isOutput=False)
    out = nc.declare_dram_parameter("out", [BL, T, C], F32, isOutput=True)

    from contextlib import ExitStack
    with nc.allow_low_precision(reason="bf16 compute, rel-err budget 2e-2"), \
         tile.TileContext(nc, trace_sim=False) as tc, ExitStack() as stk:
        sing = stk.enter_context(tc.tile_pool(name="sing", bufs=1))
        ident = sing.tile([128, 128], BF16, tag="ident")
        make_identity(nc, ident[:])
        ones = sing.tile([128, 64], BF16, tag="ones")
        nc.vector.memset(ones[:], 1.0)

        # long-lived per-batch tensors
        qT = [[sing.tile([128, T], BF16, tag=f"qT{b}_{cc}", name=f"qT{b}_{cc}") for cc in range(6)] for b in range(BL)]
        kT = [[sing.tile([128, T], BF16, tag=f"kT{b}_{cc}", name=f"kT{b}_{cc}") for cc in range(6)] for b in range(BL)]
        # v' per t-tile: 12 heads x (64 vals + 1 ones col) = 780 cols
        vp = [[sing.tile([128, 780], BF16, tag=f"vp{b}_{tk}", name=f"vp{b}_{tk}") for tk in range(9)] for b in range(BL)]
        oT = [[sing.tile([64, T], BF16, tag=f"oT{h}", bufs=1, name=f"oT{b}_{h}") for h in range(12)] for b in range(BL)]
        pw_sb = [sing.tile([64, C], BF16, tag=f"pw{h}", name=f"pw{h}") for h in range(12)]
        for h in range(12):
            nc.sync.dma_start(pw_sb[h][:], pwt[h])

        # ---------------- phase 1+2: transpose x, conv, build qT/kT/vp -------------
        with tc.tile_pool(name="cwt", bufs=1) as cwt, \
             tc.tile_pool(name="ld", bufs=2) as ld, \
             tc.tile_pool(name="xtp", bufs=7) as xtp, \
             tc.tile_pool(name="tp_ps", bufs=4, space="PSUM") as tp_ps, \
             tc.tile_pool(name="cv_ps", bufs=2, space="PSUM") as cv_ps:
            dsb = [[[cwt.tile([128, 128], BF16, tag=f"d{q}_{cc}_{t}", name=f"d{q}_{cc}_{t}") for t in range(9)]
                    for cc in range(6)] for q in range(3)]
            bsb = [[cwt.tile([128, 1], F32, tag=f"b{q}_{cc}", name=f"b{q}_{cc}") for cc in range(6)] for q in range(3)]
            for q in range(3):
                for cc in range(6):
                    for t in range(9):
                        nc.sync.dma_start(dsb[q][cc][t][:], diag[q, cc, t])
                    nc.sync.dma_start(bsb[q][cc][:], biasp[q, cc].rearrange("(a b) -> a b", b=1))

            for b in range(BL):
                # cls token, both orientations
                clsf = ld.tile([1, C], F32, tag="clsf")
                nc.sync.dma_start(clsf[:], x[b, 0:1, :])
                clsb = ld.tile([1, C], BF16, tag="clsb")
                nc.any.tensor_copy(clsb[:], clsf[:])
                clscf = ld.tile([128, 6], F32, tag="clscf")
                nc.sync.dma_start(clscf[:], x[b, 0, :].rearrange("(cc p) -> p cc", p=128))
                clscb = ld.tile([128, 6], BF16, tag="clscb")
                nc.any.tensor_copy(clscb[:], clscf[:])

                xT = [xtp.tile([128, 1156], BF16, tag="xtp", name=f"xT{i}") for i in range(6)]
                for cc in range(6):
                    nc.vector.memset(xT[cc][:], 0.0)
                    nc.any.tensor_copy(qT[b][cc][:, 0:1], clscb[:, cc:cc + 1])
                    nc.any.tensor_copy(kT[b][cc][:, 0:1], clscb[:, cc:cc + 1])
                for tk in range(9):
                    nc.vector.memset(
                        vp[b][tk][:].rearrange("p (h d) -> p h d", d=65)[:, :, 64:65], 1.0)
                for h in range(12):
                    nc.any.tensor_copy(
                        vp[b][0][0:1, h * 65:h * 65 + 64], clsb[:, h * 64:(h + 1) * 64])

                # transpose x body into padded [34,34] layout
                for tt in range(8):
                    xn = ld.tile([128, C], F32, tag="xn")
                    nc.sync.dma_start(xn[:], x[b, 1 + tt * 128:1 + (tt + 1) * 128, :])
                    xb = ld.tile([128, C], BF16, tag="xb")
                    nc.any.tensor_copy(xb[:], xn[:])
                    for cc in range(6):
                        pt = tp_ps.tile([128, 128], BF16, tag="tp")
                        nc.tensor.transpose(pt[:], xb[:, cc * 128:(cc + 1) * 128], ident[:])
                        dst = xT[cc][:].rearrange("p (r w) -> p r w", w=34)[
                            :, 1 + tt * 4:1 + tt * 4 + 4, 1:33]
                        nc.any.tensor_copy(dst, pt[:].rearrange("p (r w) -> p r w", w=32))

                # conv: 9 diag matmuls accumulated in PSUM
                for q in range(3):
                    for cc in range(6):
                        cp = cv_ps.tile([128, 1024], F32, tag="cv")
                        xv = xT[cc][:].rearrange("p (r w) -> p r w", w=34)
                        for t in range(9):
                            dy, dx = t // 3, t % 3
                            for ch in range(2):
                                rhs = xv[:, dy + ch * 16:dy + ch * 16 + 16, dx:dx + 32]
                                nc.tensor.matmul(
                                    cp[:, ch * 512:(ch + 1) * 512], dsb[q][cc][t][:], rhs,
                                    start=(t == 0), stop=(t == 8))
                        if q < 2:
                            dstT = (qT if q == 0 else kT)[b][cc]
                            nc.any.tensor_scalar_add(dstT[:, 1:T], cp[:], bsb[q][cc][:])
                        else:
                            vt = ld.tile([128, 1024], BF16, tag="vt")
                            nc.any.tensor_scalar_add(vt[:], cp[:], bsb[2][cc][:])
                            for g in range(1, 9):
                                pt = tp_ps.tile([128, 128], BF16, tag="tp")
                                nc.tensor.transpose(
                                    pt[:], vt[:, (g - 1) * 128:g * 128], ident[:])
                                dst = vp[b][g][:, 2 * cc * 65:(2 * cc + 2) * 65].rearrange(
                                    "p (h d) -> p h d", d=65)[:, :, 0:64]
                                nc.any.tensor_copy(
                                    dst, pt[:].rearrange("p (h d) -> p h d", d=64))

        # ---------------- phase 3+4 per batch: attention then projection ------------
        for b in range(BL):
            with tc.tile_pool(name=f"sc{b}", bufs=1, space="PSUM") as scp, \
                 tc.tile_pool(name=f"op{b}", bufs=1, space="PSUM") as opp, \
                 tc.tile_pool(name=f"ea{b}", bufs=3) as eap:
                for h in range(12):
                    cc, r0 = h // 2, (h % 2) * 64
                    ops = opp.tile([65, T], F32, tag="o")
                    for g in range(9):
                        tsz = 1 if g == 0 else 128
                        c0 = 0 if g == 0 else 1 + (g - 1) * 128
                        sc = scp.tile([128, T], F32, tag="sc")
                        for (l0, l1) in LCH:
                            nc.tensor.matmul(
                                sc[0:tsz, l0:l1],
                                kT[b][cc][r0:r0 + 64, c0:c0 + tsz],
                                qT[b][cc][r0:r0 + 64, l0:l1], start=True, stop=True)
                        E = eap.tile([128, T], BF16, tag="E")
                        nc.scalar.activation(
                            E[0:tsz, :], sc[0:tsz, :],
                            mybir.ActivationFunctionType.Exp, scale=SCALE)
                        for (l0, l1) in LCH:
                            nc.tensor.matmul(
                                ops[:, l0:l1], vp[b][g][0:tsz, h * 65:(h + 1) * 65],
                                E[0:tsz, l0:l1], start=(g == 0), stop=(g == 8))
                    # divide numerator rows by denominator row (row 64)
                    rc = eap.tile([65, T], BF16, tag="rc")
                    nc.vector.reciprocal(rc[64:65, :], ops[64:65, :])
                    rb = scp.tile([128, T], F32, tag="sc")
                    for (l0, l1) in LCH:
                        nc.tensor.matmul(rb[0:64, l0:l1], ones[64:65, :],
                                         rc[64:65, l0:l1], start=True, stop=True)
                    rbs = eap.tile([64, T], BF16, tag="rb")
                    nc.any.tensor_copy(rbs[:], rb[0:64, :])
                    nc.vector.tensor_mul(oT[b][h][:], ops[0:64, :], rbs[:])

            with tc.tile_pool(name=f"pr{b}", bufs=2, space="PSUM") as prp, \
                 tc.tile_pool(name=f"po{b}", bufs=3) as pop:
                for lt in range(9):
                    lsz = 128 if lt < 8 else 1
                    pp = prp.tile([128, C], F32, tag="pr")
                    for h in range(12):
                        for (e0, e1) in [(0, 512), (512, 768)]:
                            nc.tensor.matmul(
                                pp[0:lsz, e0:e1], oT[b][h][:, lt * 128:lt * 128 + lsz],
                                pw_sb[h][:, e0:e1], start=(h == 0), stop=(h == 11))
                    ob = pop.tile([128, C], F32, tag="po")
                    nc.any.tensor_copy(ob[0:lsz, :], pp[0:lsz, :])
                    nc.sync.dma_start(out[b, lt * 128:lt * 128 + lsz, :], ob[0:lsz, :])
    return nc


_CACHE = {}


def _get_nc():
    if "nc" not in _CACHE:
        nc = bacc.Bacc("TRN2", target_bir_lowering=False, debug=False,
                       enable_asserts=False, num_devices=NCORES)
        _build(nc)
        nc.compile()
        _CACHE["nc"] = nc
    return _CACHE["nc"]


def _prep_weights(w, g, bb, m, v):
    s = (np.asarray(g) / np.sqrt(np.asarray(v) + EPS)).astype(np.float32)
    w9 = np.asarray(w).reshape(C, 9).astype(np.float32) * s[:, None]
    bias = (np.asarray(bb) - np.asarray(m) * s).astype(np.float32)
    return w9, bias


def kernel(x, w_q, bn_q_g, bn_q_b, bn_q_m, bn_q_v,
           w_k, bn_k_g, bn_k_b, bn_k_m, bn_k_v,
           w_v, bn_v_g, bn_v_b, bn_v_m, bn_v_v,
           proj_w, proj_b, h, w, **_):
    x = np.asarray(x, dtype=np.float32)
    diag = np.zeros((3, 6, 9, 128, 128), dtype=ml_dtypes.bfloat16)
    biasp = np.zeros((3, 6, 128), dtype=np.float32)
    idx = np.arange(128)
    for q, (wt, gg, bb2, mm, vv) in enumerate([
            (w_q, bn_q_g, bn_q_b, bn_q_m, bn_q_v),
            (w_k, bn_k_g, bn_k_b, bn_k_m, bn_k_v),
            (w_v, bn_v_g, bn_v_b, bn_v_m, bn_v_v)]):
        w9, bias = _prep_weights(wt, gg, bb2, mm, vv)
        for cc in range(6):
            for t in range(9):
                diag[q, cc, t, idx, idx] = w9[cc * 128:(cc + 1) * 128, t].astype(
                    ml_dtypes.bfloat16)
            biasp[q, cc] = bias[cc * 128:(cc + 1) * 128]
    pwt = np.ascontiguousarray(
        np.asarray(proj_w, np.float32).T.reshape(12, 64, C)).astype(ml_dtypes.bfloat16)

    nc = _get_nc()
    in_maps = []
    for ci in range(NCORES):
        in_maps.append({
            "x": np.ascontiguousarray(x[ci * BL:(ci + 1) * BL]),
            "diag": diag, "biasp": biasp, "pwt": pwt,
        })
    res = run_bass_kernel_spmd(nc, in_maps, core_ids=list(range(NCORES)))
    outs = [res.results[ci]["out"] for ci in range(NCORES)]
    full = np.concatenate(outs, axis=0).astype(np.float32)
    full += np.asarray(proj_b, np.float32)[None, None, :]
    return full
